# revision 5
# baseline (speedup 1.0000x reference)
"""Trainium2 Bass kernel for a dense transformer block (pre-LN MHA + MLP).

Sharding: pure data parallel — batch (8) maps 1:1 onto the 8 NeuronCores.
Each core runs the full block on one [1024, 1024] slice with replicated
weights (host-cast to fp16 for the tensor engine; fp32 residual path).

Self-contained: hardcodes all shapes from the problem spec.
"""

import numpy as np

import concourse.bass as bass
import concourse.tile as tile
from concourse import bacc, mybir
from concourse.bass import ts
from concourse.bass_utils import run_bass_kernel_spmd
from concourse.masks import make_identity

F32 = mybir.dt.float32
F16 = mybir.dt.float16
AF = mybir.ActivationFunctionType
ALU = mybir.AluOpType

P = 128          # partitions
N = 1024         # tokens per core
D = 1024         # model dim
KC = D // P      # 8 contraction chunks of 128
HEADS = 16
HD = 64          # head dim
HID = 4096
EPS = 1e-6
NT = N // 512    # 2 chunks of 512 tokens
MT = N // P      # 8 token tiles of 128
SCALE = HD ** -0.5


def build_block(apply_c1: bool, apply_bfc2: bool):
    nc = bacc.Bacc("TRN2", target_bir_lowering=False, debug=False, num_devices=8)

    x_d = nc.dram_tensor("x", [N, D], F32, kind="ExternalInput")
    wqkv_d = nc.dram_tensor("w_qkv", [D, 3 * D], F16, kind="ExternalInput")
    wproj_d = nc.dram_tensor("w_proj", [D, D], F16, kind="ExternalInput")
    wfc1_d = nc.dram_tensor("w_fc1", [D, HID], F16, kind="ExternalInput")
    wfc2_d = nc.dram_tensor("w_fc2", [HID, D], F16, kind="ExternalInput")
    bqkv_d = nc.dram_tensor("b_qkv", [3 * D], F32, kind="ExternalInput")
    bfc1_d = nc.dram_tensor("b_fc1", [HID], F32, kind="ExternalInput")
    ln1s_d = nc.dram_tensor("ln1_scale", [D], F32, kind="ExternalInput")
    ln1b_d = nc.dram_tensor("ln1_bias", [D], F32, kind="ExternalInput")
    ln2s_d = nc.dram_tensor("ln2_scale", [D], F32, kind="ExternalInput")
    ln2b_d = nc.dram_tensor("ln2_bias", [D], F32, kind="ExternalInput")
    c1_d = nc.dram_tensor("c1", [D], F32, kind="ExternalInput") if apply_c1 else None
    bfc2_d = (
        nc.dram_tensor("b_fc2c", [D], F32, kind="ExternalInput") if apply_bfc2 else None
    )
    y_d = nc.dram_tensor("y", [N, D], F32, kind="ExternalOutput")

    # [(kc p), n] -> [p, kc, n] views for weight loads (lhsT layout)
    wqkv_v = wqkv_d.ap().rearrange("(kc p) n -> p kc n", p=P)
    wproj_v = wproj_d.ap().rearrange("(kc p) n -> p kc n", p=P)
    wfc1_v = wfc1_d.ap().rearrange("(kc p) n -> p kc n", p=P)
    wfc2_v = wfc2_d.ap().rearrange("(kc p) n -> p kc n", p=P)

    from contextlib import ExitStack

    with tile.TileContext(nc) as tc, ExitStack() as ctx:
        ep = ctx.enter_context
        constp = ep(tc.tile_pool(name="const", bufs=1))
        xload = ep(tc.tile_pool(name="xload", bufs=2))
        x1p = ep(tc.tile_pool(name="x1", bufs=1))
        htmpp = ep(tc.tile_pool(name="htmp", bufs=2))
        hTp = ep(tc.tile_pool(name="hT", bufs=1))
        qTp = ep(tc.tile_pool(name="qT", bufs=1))
        kTp = ep(tc.tile_pool(name="kT", bufs=1))
        vp = ep(tc.tile_pool(name="vv", bufs=1))
        oTp = ep(tc.tile_pool(name="oT", bufs=1))
        probsp = ep(tc.tile_pool(name="probs", bufs=2))
        wp = ep(tc.tile_pool(name="w", bufs=2))
        statsp = ep(tc.tile_pool(name="stats", bufs=4))
        recipp = ep(tc.tile_pool(name="recip", bufs=2))
        otmpp = ep(tc.tile_pool(name="otmp", bufs=2))
        sumsp = ep(tc.tile_pool(name="sums", bufs=4, space="DRAM"))
        pmp = ep(tc.tile_pool(name="pm", bufs=4, space="PSUM"))
        ptp = ep(tc.tile_pool(name="pt", bufs=2, space="PSUM"))
        pavp = ep(tc.tile_pool(name="pav", bufs=2, space="PSUM"))
        if True:
            # ---- constants ----
            ident = constp.tile([P, P], F16)
            make_identity(nc, ident[:])
            eps_t = constp.tile([P, 1], F32)
            nc.vector.memset(eps_t[:], EPS)
            ln1s = constp.tile([P, KC], F32)
            nc.sync.dma_start(ln1s[:], ln1s_d.ap().rearrange("(k p) -> p k", p=P))
            ln1b = constp.tile([P, KC], F32)
            nc.sync.dma_start(ln1b[:], ln1b_d.ap().rearrange("(k p) -> p k", p=P))
            ln2s = constp.tile([P, KC], F32)
            nc.sync.dma_start(ln2s[:], ln2s_d.ap().rearrange("(k p) -> p k", p=P))
            ln2b = constp.tile([P, KC], F32)
            nc.sync.dma_start(ln2b[:], ln2b_d.ap().rearrange("(k p) -> p k", p=P))
            bqk = constp.tile([P, 16], F32)  # q,k bias columns (out_c 0..2047)
            bqkv_v = bqkv_d.ap().rearrange("(m p) -> p m", p=P)
            nc.sync.dma_start(bqk[:], bqkv_v[:, 0:16])
            bfc1 = constp.tile([P, HID // P], F32)
            nc.sync.dma_start(bfc1[:], bfc1_d.ap().rearrange("(m p) -> p m", p=P))
            if apply_c1:
                c1row = constp.tile([P, D], F32)
                src = c1_d.ap()
                nc.sync.dma_start(
                    c1row[:],
                    bass.AP(tensor=src.tensor, offset=src.offset, ap=[[0, P], [1, D]]),
                )
            if apply_bfc2:
                b2row = constp.tile([P, D], F32)
                src = bfc2_d.ap()
                nc.sync.dma_start(
                    b2row[:],
                    bass.AP(tensor=src.tensor, offset=src.offset, ap=[[0, P], [1, D]]),
                )

            hT = hTp.tile([P, KC, N], F16, tag="hT")

            def layer_norm_to_hT(src_ap, out_hT, s_cols, b_cols, mt):
                """LN over free dim of src [128, 1024]; write transposed fp16
                into out_hT[:, kc, mt*128:...] with scale/bias fused."""
                st = statsp.tile([P, 2, 6], F32)
                xr = src_ap.rearrange("p (a b) -> p a b", b=512)
                nc.vector.bn_stats(st[:, 0, :], xr[:, 0, :])
                nc.vector.bn_stats(st[:, 1, :], xr[:, 1, :])
                mv = statsp.tile([P, 2], F32)
                nc.vector.bn_aggr(mv[:], st[:])
                rstd = statsp.tile([P, 1], F32)
                nc.scalar.activation(rstd[:], mv[:, 1:2], AF.Sqrt, bias=eps_t[:])
                nc.vector.reciprocal(rstd[:], rstd[:])
                h = htmpp.tile([P, D], F16)
                nc.vector.tensor_scalar(
                    out=h[:], in0=src_ap, scalar1=mv[:, 0:1], scalar2=rstd[:],
                    op0=ALU.subtract, op1=ALU.mult,
                )
                for kc in range(KC):
                    pt_t = ptp.tile([P, P], F16)
                    nc.tensor.transpose(pt_t[:], h[:, ts(kc, P)], ident[:])
                    nc.vector.tensor_scalar(
                        out=out_hT[:, kc, ts(mt, P)], in0=pt_t[:],
                        scalar1=s_cols[:, kc : kc + 1], scalar2=b_cols[:, kc : kc + 1],
                        op0=ALU.mult, op1=ALU.add,
                    )

            # ---- phase 1: LN1 + transpose ----
            for mt in range(MT):
                x_t = xload.tile([P, D], F32)
                nc.sync.dma_start(x_t[:], x_d.ap()[ts(mt, P), :])
                layer_norm_to_hT(x_t[:], hT, ln1s, ln1b, mt)

            # ---- phase 2: qkv ----
            qT = qTp.tile([P, KC, N], F16, tag="qT")
            kT = kTp.tile([P, KC, N], F16, tag="kT")
            v_sb = vp.tile([P, MT, HEADS * (HD + 1)], F16, tag="vv")

            for piece in range(2):  # q then k
                w_t = wp.tile([P, KC, 1024], F16, tag="w")
                nc.sync.dma_start(w_t[:], wqkv_v[:, :, ts(piece, 1024)])
                dst = qT if piece == 0 else kT
                for mc in range(8):
                    mcg = piece * 8 + mc
                    for nt in range(NT):
                        ps = pmp.tile([P, 512], F32)
                        for kc in range(KC):
                            nc.tensor.matmul(
                                ps[:], w_t[:, kc, ts(mc, P)], hT[:, kc, ts(nt, 512)],
                                start=(kc == 0), stop=(kc == KC - 1),
                            )
                        nc.vector.tensor_scalar(
                            out=dst[:, mc, ts(nt, 512)], in0=ps[:],
                            scalar1=bqk[:, mcg : mcg + 1], scalar2=None, op0=ALU.add,
                        )

            # v (token-major), with ones column per head at stride 65
            w_t = wp.tile([P, KC, 1024], F16, tag="w")
            nc.sync.dma_start(w_t[:], wqkv_v[:, :, 2048:3072])
            for mt in range(MT):
                v_row = v_sb[:, mt, :].rearrange("p (h c) -> p h c", c=HD + 1)
                nc.vector.memset(v_row[:, :, HD : HD + 1], 1.0)
                for nv in range(2):
                    ps = pmp.tile([P, 512], F32)
                    for kc in range(KC):
                        nc.tensor.matmul(
                            ps[:], hT[:, kc, ts(mt, P)], w_t[:, kc, ts(nv, 512)],
                            start=(kc == 0), stop=(kc == KC - 1),
                        )
                    nc.vector.tensor_copy(
                        out=v_row[:, 8 * nv : 8 * nv + 8, 0:HD],
                        in_=ps[:].rearrange("p (h c) -> p h c", c=HD),
                    )

            # w_proj load early (streams behind attention)
            wproj_t = wp.tile([P, KC, 1024], F16, tag="w")
            nc.sync.dma_start(wproj_t[:], wproj_v[:])

            # ---- phase 3: attention ----
            oT = oTp.tile([P, KC, N], F16, tag="oT")
            for h in range(HEADS):
                mc_h = h // 2
                pr = (h % 2) * HD
                for nq in range(NT):
                    probs = probsp.tile([P, KC, 512], F16)
                    for mk in range(MT):
                        ps = pmp.tile([P, 512], F32)
                        nc.tensor.matmul(
                            ps[:],
                            kT[pr : pr + HD, mc_h, ts(mk, P)],
                            qT[pr : pr + HD, mc_h, ts(nq, 512)],
                            start=True, stop=True,
                        )
                        nc.scalar.activation(
                            probs[:, mk, :], ps[:], AF.Exp, scale=SCALE
                        )
                    pav = pavp.tile([P, 512], F32)
                    for mk in range(MT):
                        nc.tensor.matmul(
                            pav[0 : HD + 1, :],
                            v_sb[:, mk, h * (HD + 1) : (h + 1) * (HD + 1)],
                            probs[:, mk, :],
                            start=(mk == 0), stop=(mk == MT - 1),
                        )
                    srow = otmpp.tile([1, 512], F32, tag="srow")
                    nc.vector.tensor_copy(srow[:], pav[HD : HD + 1, :])
                    sums_t = sumsp.tile([1, 512], F32)
                    nc.sync.dma_start(sums_t[:], srow[:])
                    rb = recipp.tile([HD, 512], F32)
                    s_ap = sums_t[:]
                    nc.sync.dma_start(
                        rb[:],
                        bass.AP(
                            tensor=s_ap.tensor, offset=s_ap.offset, ap=[[0, HD], [1, 512]]
                        ),
                    )
                    nc.vector.reciprocal(rb[:], rb[:])
                    o_t = otmpp.tile([HD, 512], F16)
                    nc.vector.tensor_mul(o_t[:], pav[0:HD, :], rb[:])
                    nc.sync.dma_start(oT[pr : pr + HD, mc_h, ts(nq, 512)], o_t[:])

            # ---- phase 4: proj + residual -> x1 ----
            x1 = x1p.tile([P, MT, D], F32)
            for mt in range(MT):
                x_t = xload.tile([P, D], F32)
                nc.sync.dma_start(x_t[:], x_d.ap()[ts(mt, P), :])
                for np_ in range(NT):
                    ps = pmp.tile([P, 512], F32)
                    for kc in range(KC):
                        nc.tensor.matmul(
                            ps[:], oT[:, kc, ts(mt, P)], wproj_t[:, kc, ts(np_, 512)],
                            start=(kc == 0), stop=(kc == KC - 1),
                        )
                    nc.vector.tensor_add(
                        x1[:, mt, ts(np_, 512)], ps[:], x_t[:, ts(np_, 512)]
                    )
                    if apply_c1:
                        nc.vector.tensor_add(
                            x1[:, mt, ts(np_, 512)],
                            x1[:, mt, ts(np_, 512)],
                            c1row[:, ts(np_, 512)],
                        )

            # ---- phase 5: LN2 + transpose ----
            h2T = hTp.tile([P, KC, N], F16, tag="hT")
            for mt in range(MT):
                layer_norm_to_hT(x1[:, mt, :], h2T, ln2s, ln2b, mt)

            # ---- phase 6: fc1 (gelu) ----
            # a1T groups g=0..3 each [128, 8, 1024] fp16, reusing attention pools
            a1_pools = [(qTp, "qT"), (kTp, "kT"), (vp, "vv"), (oTp, "oT")]
            a1 = []
            for g in range(4):
                pool_g, tag_g = a1_pools[g]
                a1_g = pool_g.tile([P, KC, N], F16, tag=tag_g)
                a1.append(a1_g)
                w1_t = wp.tile([P, KC, 1024], F16, tag="w")
                nc.sync.dma_start(w1_t[:], wfc1_v[:, :, ts(g, 1024)])
                for mh in range(8):
                    for nt in range(NT):
                        ps = pmp.tile([P, 512], F32)
                        for kc in range(KC):
                            nc.tensor.matmul(
                                ps[:], w1_t[:, kc, ts(mh, P)], h2T[:, kc, ts(nt, 512)],
                                start=(kc == 0), stop=(kc == KC - 1),
                            )
                        mhg = g * 8 + mh
                        nc.scalar.activation(
                            a1[g][:, mh, ts(nt, 512)], ps[:], AF.Gelu_apprx_tanh,
                            bias=bfc1[:, mhg : mhg + 1],
                        )

            # ---- phase 7: fc2 + residual -> y ----
            # stream w2 group-by-group; accumulate partial products into x1
            for g in range(4):
                w2_t = wp.tile([P, KC, 1024], F16, tag="w")
                nc.sync.dma_start(w2_t[:], wfc2_v[:, ts(g, KC), :])
                for mt in range(MT):
                    for ncol in range(NT):
                        ps = pmp.tile([P, 512], F32)
                        for kc in range(KC):
                            nc.tensor.matmul(
                                ps[:], a1[g][:, kc, ts(mt, P)],
                                w2_t[:, kc, ts(ncol, 512)],
                                start=(kc == 0), stop=(kc == KC - 1),
                            )
                        nc.vector.tensor_add(
                            x1[:, mt, ts(ncol, 512)], ps[:], x1[:, mt, ts(ncol, 512)]
                        )
            for mt in range(MT):
                if apply_bfc2:
                    nc.vector.tensor_add(
                        x1[:, mt, :], x1[:, mt, :], b2row[:]
                    )
                nc.sync.dma_start(y_d.ap()[ts(mt, P), :], x1[:, mt, :])

    nc.compile()
    return nc


_cache = {}


def _get_nc(apply_c1, apply_bfc2):
    key = (apply_c1, apply_bfc2)
    if key not in _cache:
        _cache[key] = build_block(apply_c1, apply_bfc2)
    return _cache[key]


def kernel(
    x, w_qkv, b_qkv, w_proj, b_proj, ln1_scale, ln1_bias,
    ln2_scale, ln2_bias, w_fc1, b_fc1, w_fc2, b_fc2,
):
    x = np.asarray(x, np.float32)
    B = x.shape[0]
    b_qkv = np.asarray(b_qkv, np.float32)
    b_v = b_qkv[2 * D :]
    # exact folds: o includes +b_v after softmax-normalize (rows sum to 1),
    # so c1 = b_v @ w_proj + b_proj is a constant row added post-proj.
    c1 = b_v.astype(np.float64) @ np.asarray(w_proj, np.float64) + np.asarray(
        b_proj, np.float64
    )
    c1 = c1.astype(np.float32)
    bfc2 = np.asarray(b_fc2, np.float32)
    apply_c1 = bool(np.any(c1 != 0))
    apply_bfc2 = bool(np.any(bfc2 != 0))

    nc = _get_nc(apply_c1, apply_bfc2)

    base = {
        "w_qkv": np.asarray(w_qkv, np.float16),
        "w_proj": np.asarray(w_proj, np.float16),
        "w_fc1": np.asarray(w_fc1, np.float16),
        "w_fc2": np.asarray(w_fc2, np.float16),
        "b_qkv": b_qkv,
        "b_fc1": np.asarray(b_fc1, np.float32),
        "ln1_scale": np.asarray(ln1_scale, np.float32),
        "ln1_bias": np.asarray(ln1_bias, np.float32),
        "ln2_scale": np.asarray(ln2_scale, np.float32),
        "ln2_bias": np.asarray(ln2_bias, np.float32),
    }
    if apply_c1:
        base["c1"] = c1
    if apply_bfc2:
        base["b_fc2c"] = bfc2

    in_maps = [dict(base, x=np.ascontiguousarray(x[i])) for i in range(B)]
    res = run_bass_kernel_spmd(nc, in_maps, core_ids=list(range(B)))
    out = np.stack([res.results[i]["y"] for i in range(B)], axis=0)
    return out.astype(np.float32)
